# revision 1
# baseline (speedup 1.0000x reference)
"""FJSP decoder kernel for Trainium2, data-parallel over batch on 8 NeuronCores.

Key algebraic restructuring: q/k/v for the flattened (job, machine) pair
s=(j,m) decompose as x[s] = xj[j] + xm[m], so the joint-axis attention
softmax factorizes exactly:

  score[s, (j',m')] = E[s,j'] + F[s,m']      (E from A,C; F from B,Dm)
  softmax_t(score) @ v = softmax_j'(E) @ vj + softmax_m'(F) @ vm

and with E[(j,m),j'] = (A[j,j'] + C[m,j'])/sqrt(QD) the row softmax of E
itself factorizes through exp(A)*exp(C), giving per head only J*J-sized
matmuls -- the [S,S] = [2000,2000] score matrix is never materialized.
The multi-head combine collapses through w2 = Wmhc @ Wshc into per-head
scalars uv = v @ w2, so the whole decoder reduces to [100,20]-shaped work:

  SE|Nj = eAT.T @ [eCT | eCT*uvj];  SF|Nm = eBT.T @ [eDT | eDT*uvm]
  score1 = (sum_h Nj/SE + Nm/SF + bias)/sqrt(D)
  p = softmax_flat(10*tanh(score1) + mask)   (tanh via exp, one ACT table)

Layout notes: heads are padded to 32-partition strips (two groups of 4
heads) so per-head K=16 contractions become legal K=32 matmuls at base
partitions {0,32,64,96}; k/v projection tiles carry 80 zero columns so
every per-head matmul runs in the same (32-row, 128-col) PE tiling mode.
All inputs are host-packed into two DRAM tensors (weights, activations)
so the kernel issues exactly two input DMAs.
"""

import math

import numpy as np

import concourse.bass as bass
import concourse.mybir as mybir
import concourse.tile as tile
from concourse.bass_utils import run_bass_kernel_spmd
from concourse.masks import make_identity

F32 = mybir.dt.float32
AF = mybir.ActivationFunctionType
OP = mybir.AluOpType
AX = mybir.AxisListType

D, H, QD = 128, 8, 16
B, J, M = 8, 100, 20
HQ = H * QD  # 128
INV_SQ = 1.0 / math.sqrt(QD)  # 0.25
SD = math.sqrt(D)

# edata column layout: [ej 0:128 | em 128:256 | mask 256:276 | smallw 276:279]
EJ0, EM0, MK0, SW0 = 0, 128, 256, 276
EDATA_W = 279

# ---------------------------------------------------------------------------
# gen3 walrus accepts one sync-wait per instruction. Tile's kernel-tail
# drain accumulates one wait per active logical processor on a single
# Drain: spread them across engines (parallel waiting). Tile's semaphore
# pass can also attach >1 wait to ordinary instructions: shed extras onto
# same-engine NoOps inserted right before the offender.
_PATCHED = False


def _install_drain_patch():
    global _PATCHED
    if _PATCHED:
        return
    from concourse.tile import ScopedClock, TileContext

    def _split_drain_and_barrier(self, tick_clock, wait_clock):
        drain_inst = self.nc.sync.drain()
        wait_clock.add_sem_waits(
            drain_inst.ins, ScopedClock({None: tick_clock.global_clock})
        )
        si = drain_inst.ins.sync_info
        waits = list(si.on_wait) if si is not None else []
        if len(waits) > 1:
            assert not si.on_update
            sems = {s.name: s for s in self.sems.allocated().values()}
            drain_inst.ins.sync_info = None
            drain_inst.wait_op(sems[waits[0].ant_name], waits[0].wait_value, "sem-ge")
            engines = [
                self.nc.scalar,
                self.nc.vector,
                self.nc.tensor,
                self.nc.gpsimd,
                self.nc.sync,
            ]
            for i, w in enumerate(waits[1:]):
                extra = engines[i % len(engines)].drain()
                extra.wait_op(sems[w.ant_name], w.wait_value, "sem-ge")
        self.nc.all_engine_barrier()
        assert self.sems is not None
        popped = self.nc._tile_sem_poison_stack.pop()
        assert popped is self._sem_poison
        self.nc.clear_and_free_semaphores(list(self.sems.allocated().values()))

    TileContext._drain_and_barrier = _split_drain_and_barrier
    _PATCHED = True


def _split_multi_waits(nc):
    import bass_rust

    ctr = 0
    for fn in nc.m.functions:
        for bb in fn.blocks:
            il = bb.instructions
            if not any(
                i.sync_info is not None and len(i.sync_info.on_wait) > 1 for i in il
            ):
                continue
            new = []
            for ins in il:
                si = ins.sync_info
                if si is not None and len(si.on_wait) > 1:
                    waits = list(si.on_wait)
                    ups = list(si.on_update)
                    for w in waits[:-1]:
                        nop = mybir.InstNoOp(name=f"I-waitsplit-{ctr}", ins=[], outs=[])
                        ctr += 1
                        nop.engine = ins.engine
                        nop.sync_info = bass_rust.SyncInfo(on_update=[], on_wait=[w])
                        new.append(nop)
                    ins.sync_info = bass_rust.SyncInfo(
                        on_update=ups, on_wait=[waits[-1]]
                    )
                new.append(ins)
            bb.instructions = new


def _chunk2(ap_slice, chunk_step):
    """Matmul rhs built from two equal column chunks `chunk_step` apart."""
    return bass.AP(
        tensor=ap_slice.tensor,
        offset=ap_slice.offset,
        ap=[ap_slice.ap[0], [chunk_step, 2], ap_slice.ap[1]],
    )


def _build():
    nc = bass.Bass()
    # wqkv[:, i, :]: 0=Wq3-job 1=Wq3-mach 2=Wk-job 3=Wk-mach 4=Wv-job
    # 5=Wv-mach 6=Wmhc
    wqkv_d = nc.dram_tensor("wqkv", [D, 7, D], F32, kind="ExternalInput")
    ed_d = nc.dram_tensor("edata", [D, EDATA_W], F32, kind="ExternalInput")
    out_d = nc.dram_tensor("out", [J, M], F32, kind="ExternalOutput")

    with tile.TileContext(nc) as tc:
        with (
            tc.tile_pool(name="persist", bufs=1) as pp,
            tc.tile_pool(name="rot", bufs=8) as rp,
            tc.tile_pool(name="ps_big", bufs=2, space="PSUM") as ps_big,
            tc.tile_pool(name="ps_s1", bufs=6, space="PSUM") as ps_s1,
        ):
            # ---- constants that gate the PE transposes ------------------
            ident = pp.tile([D, D], F32, tag="ident")
            make_identity(nc, ident)

            # ---- the two input DMAs -------------------------------------
            ed_sb = pp.tile([D, EDATA_W], F32, tag="edata")
            nc.sync.dma_start(out=ed_sb, in_=ed_d[:])
            wqkv_sb = pp.tile([D, 7, D], F32, tag="wqkv")
            nc.sync.dma_start(out=wqkv_sb, in_=wqkv_d[:])

            ej_v = ed_sb[0:J, EJ0 : EJ0 + D]
            em_v = ed_sb[0:M, EM0 : EM0 + D]
            mask_v = ed_sb[0:J, MK0 : MK0 + M]
            bmhc_v = ed_sb[:, SW0 : SW0 + 1]
            wshc_v = ed_sb[:, SW0 + 1 : SW0 + 2]
            bshc_v = ed_sb[0:1, SW0 + 2 : SW0 + 3]

            # k/v projection tiles get 80 zero cols (120:200) so machine-
            # side per-head matmuls run with M=100 (128-col PE mode)
            pT_sb = {}
            for nm in ("q", "k", "v"):
                for grp in range(2):
                    w = 120 if nm == "q" else 200
                    sb = pp.tile([D, w], F32, tag=f"{nm}T{grp}")
                    if nm != "q":
                        nc.gpsimd.memset(sb[:, 120:200], 0.0)
                    pT_sb[(nm, grp)] = sb

            ones_sb = pp.tile([D, D], F32, tag="ones")
            nc.gpsimd.memset(ones_sb, 1.0)

            # padded weights: head h -> 32-strip 32g..32g+16 (g = h%4) in
            # group A (h<4) / B (h>=4); the other 16 lanes zero.
            wpad = pp.tile([D, 12, D], F32, tag="wpad")
            wpad_idx = {}
            idx = 0
            for nm_i, nm in enumerate(("q", "k", "v")):
                for half in range(2):
                    for grp in range(2):
                        wpad_idx[(nm, half, grp)] = idx
                        eng = nc.vector if nm == "q" else nc.gpsimd
                        tv = wpad[:, idx, :].rearrange("p (g c) -> p g c", c=32)
                        eng.memset(tv[:, :, 16:32], 0.0)
                        src = wqkv_sb[
                            :, nm_i * 2 + half, grp * 64 : (grp + 1) * 64
                        ].rearrange("p (g c) -> p g c", c=16)
                        eng.tensor_copy(out=tv[:, :, 0:16], in_=src)
                        idx += 1

            # ---- PE transposes (wmhcT first: longest downstream chain) --
            wmhcT_ps = ps_big.tile([D, 320], F32, tag="big")
            nc.tensor.transpose(wmhcT_ps[:, 0:HQ], wqkv_sb[:, 6, :], ident)
            wmhcT_sb = pp.tile([D, HQ], F32, tag="wmhcT")
            nc.scalar.copy(out=wmhcT_sb, in_=wmhcT_ps[:, 0:HQ])

            ejT_ps = ps_big.tile([D, 320], F32, tag="big")
            nc.tensor.transpose(ejT_ps[:, 0:J], ej_v, ident[0:J, 0:J])
            ejT_sb = pp.tile([D, J], F32, tag="ejT")
            nc.scalar.copy(out=ejT_sb, in_=ejT_ps[:, 0:J])

            emT_ps = ps_big.tile([D, 320], F32, tag="big")
            nc.tensor.transpose(emT_ps[:, 0:M], em_v, ident[0:M, 0:M])
            emT_sb = pp.tile([D, M], F32, tag="emT")
            nc.scalar.copy(out=emT_sb, in_=emT_ps[:, 0:M])

            # WmhcT with columns in padded-head layout, per group
            wmhcPT = pp.tile([D, 2, D], F32, tag="wmhcPT")
            for grp in range(2):
                tv = wmhcPT[:, grp, :].rearrange("p (g c) -> p g c", c=32)
                nc.gpsimd.memset(tv[:, :, 16:32], 0.0)
                src = wmhcT_sb[:, grp * 64 : (grp + 1) * 64].rearrange(
                    "p (g c) -> p g c", c=16
                )
                nc.gpsimd.tensor_copy(out=tv[:, :, 0:16], in_=src)

            # ---- 128x128 mode: projections, w2pad, bias -----------------
            for nm in ("v", "k", "q"):
                for grp in range(2):
                    ps = ps_big.tile([D, 320], F32, tag="big")
                    nc.tensor.matmul(
                        out=ps[:, 0:J],
                        lhsT=wpad[:, wpad_idx[(nm, 0, grp)], :],
                        rhs=ejT_sb,
                    )
                    nc.tensor.matmul(
                        out=ps[:, J : J + M],
                        lhsT=wpad[:, wpad_idx[(nm, 1, grp)], :],
                        rhs=emT_sb,
                    )
                    sb = pT_sb[(nm, grp)]
                    if nm == "v":
                        nc.vector.tensor_copy(out=sb[:, 0:120], in_=ps[:, 0:120])
                    else:
                        nc.scalar.copy(out=sb[:, 0:120], in_=ps[:, 0:120])

            w2pad_sb = []
            for grp in range(2):
                ps = ps_big.tile([D, 320], F32, tag="big")
                nc.tensor.matmul(
                    out=ps[:, 0:1], lhsT=wmhcPT[:, grp, :], rhs=wshc_v
                )
                sb = pp.tile([D, 1], F32, tag=f"w2pad{grp}")
                nc.vector.tensor_copy(out=sb, in_=ps[:, 0:1])
                w2pad_sb.append(sb)

            # bias_const = b_mhc @ Wshc + b_shc, broadcast over J partitions
            bw = pp.tile([D, 1], F32, tag="bw")
            nc.vector.tensor_mul(out=bw, in0=bmhc_v, in1=wshc_v)
            nc.vector.tensor_add(out=bw[0:1, 0:1], in0=bw[0:1, 0:1], in1=bshc_v)
            bias_ps = ps_big.tile([D, 320], F32, tag="big")
            nc.tensor.matmul(out=bias_ps[0:J, 0:1], lhsT=ones_sb[:, 0:J], rhs=bw)
            biasb = pp.tile([J, 1], F32, tag="biasb")
            nc.scalar.mul(out=biasb, in_=bias_ps[0:J, 0:1], mul=2.0 / SD)

            # ---- (32,128) mode: uv vectors + per-head products ----------
            uvj_ps = ps_big.tile([D, 320], F32, tag="big")
            uvm_ps = ps_big.tile([D, 320], F32, tag="big")
            for h in range(H):
                grp, g = divmod(h, 4)
                vt = pT_sb[("v", grp)]
                nc.tensor.matmul(
                    out=uvj_ps[0:J, h : h + 1],
                    lhsT=vt[32 * g : 32 * g + 32, 0:J],
                    rhs=w2pad_sb[grp][32 * g : 32 * g + 32, :],
                    tile_position=(32 * g, 0),
                )
                nc.tensor.matmul(
                    out=uvm_ps[0:J, h : h + 1],
                    lhsT=vt[32 * g : 32 * g + 32, 100:200],
                    rhs=w2pad_sb[grp][32 * g : 32 * g + 32, :],
                    tile_position=(32 * g, 0),
                )
            uvj_sb = pp.tile([J, H], F32, tag="uvj")
            nc.vector.tensor_copy(out=uvj_sb, in_=uvj_ps[0:J, 0:H])
            uvm_sb = pp.tile([M, H], F32, tag="uvm")
            nc.vector.tensor_copy(out=uvm_sb, in_=uvm_ps[0:M, 0:H])

            # per head: o_ps = [AT|CT | BT|DT(+zeros)], one exp, uv scales
            f_ps = ps_big.tile([D, 8, 40], F32, tag="big")
            s_ps = ps_big.tile([D, 8, 40], F32, tag="big")
            eE = []
            for h in range(H):
                grp, g = divmod(h, 4)
                kt, qt = pT_sb[("k", grp)], pT_sb[("q", grp)]
                ps = ps_s1.tile([D, 240], F32, tag="s1")
                nc.tensor.matmul(
                    out=ps[0:J, 0:120],
                    lhsT=kt[32 * g : 32 * g + 32, 0:J],
                    rhs=qt[32 * g : 32 * g + 32, 0:120],
                    tile_position=(32 * g, 0),
                )
                nc.tensor.matmul(
                    out=ps[0:J, 120:240],
                    lhsT=kt[32 * g : 32 * g + 32, 100:200],
                    rhs=qt[32 * g : 32 * g + 32, 0:120],
                    tile_position=(32 * g, 0),
                )
                e1 = rp.tile([D, 280], F32, tag="eE")
                nc.scalar.activation(
                    out=e1[0:J, 0:240], in_=ps[0:J, 0:240], func=AF.Exp, scale=INV_SQ
                )
                nc.vector.tensor_scalar_mul(
                    out=e1[0:J, 240:260],
                    in0=e1[0:J, 100:120],
                    scalar1=uvj_sb[:, h : h + 1],
                )
                nc.vector.tensor_scalar_mul(
                    out=e1[0:M, 260:280],
                    in0=e1[0:M, 220:240],
                    scalar1=uvm_sb[:, h : h + 1],
                )
                eE.append(e1)
                # mm4: [SF|Nm] = eBT.T @ [eDT | eDT*uvm]   (K=20)
                nc.tensor.matmul(
                    out=f_ps[0:J, h, :],
                    lhsT=e1[0:M, 120:220],
                    rhs=_chunk2(e1[0:M, 220:240], 40),
                )
                # mm3: [SE|Nj] = eAT.T @ [eCT | eCT*uvj]   (K=100)
                nc.tensor.matmul(
                    out=s_ps[0:J, h, :],
                    lhsT=e1[0:J, 0:J],
                    rhs=_chunk2(e1[0:J, 100:120], 140),
                )

            def pmh(ap3):  # [p, h, m] -> [p, m, h]
                return ap3.rearrange("p h m -> p m h")

            # F-side combine first
            rF = pp.tile([J, M, H], F32, tag="rF")
            nc.vector.reciprocal(out=rF, in_=pmh(f_ps[0:J, :, 0:M]))
            d2 = pp.tile([J, M, H], F32, tag="d2")
            nc.vector.tensor_mul(out=d2, in0=pmh(f_ps[0:J, :, M : 2 * M]), in1=rF)

            # ---- combine: sum_h Nj/SE + Nm/SF ---------------------------
            rE = pp.tile([J, M, H], F32, tag="rE")
            nc.vector.reciprocal(out=rE, in_=pmh(s_ps[0:J, :, 0:M]))
            c8 = pp.tile([J, M, H], F32, tag="c8")
            nc.vector.scalar_tensor_tensor(
                out=c8, in0=pmh(s_ps[0:J, :, M : 2 * M]), scalar=1.0, in1=rE,
                op0=OP.mult, op1=OP.mult,
            )
            nc.vector.tensor_add(out=c8, in0=c8, in1=d2)
            c1 = pp.tile([J, M], F32, tag="c1")
            nc.vector.reduce_sum(out=c1, in_=c8, axis=AX.X)

            # tanh chain via exp (no ACT table switch):
            # logits ~ mask - 20/(exp(2*(c1+bias)/sqrt(D)) + 1)  (+const)
            u = pp.tile([J, M], F32, tag="u")
            nc.scalar.activation(out=u, in_=c1, func=AF.Exp, scale=2.0 / SD, bias=biasb)
            t1 = pp.tile([J, M], F32, tag="t1")
            nc.scalar.add(out=t1, in_=u, add=1.0)
            r = pp.tile([J, M], F32, tag="r")
            nc.vector.reciprocal(out=r, in_=t1)
            arg = pp.tile([J, M], F32, tag="arg")
            nc.vector.scalar_tensor_tensor(
                out=arg, in0=r, scalar=-20.0, in1=mask_v, op0=OP.mult, op1=OP.add
            )
            e_sb = pp.tile([J, M], F32, tag="e")
            s_row = pp.tile([J, 1], F32, tag="srow")
            nc.scalar.activation(
                out=e_sb, in_=arg, func=AF.Exp, scale=1.0, accum_out=s_row
            )
            totb_ps = ps_big.tile([D, 320], F32, tag="big")
            nc.tensor.matmul(out=totb_ps[0:J, 0:1], lhsT=ones_sb[0:J, 0:J], rhs=s_row)
            rtot = pp.tile([J, 1], F32, tag="rtot")
            nc.vector.reciprocal(out=rtot, in_=totb_ps[0:J, 0:1])
            out_t = pp.tile([J, M], F32, tag="outt")
            nc.vector.tensor_scalar_mul(out=out_t, in0=e_sb, scalar1=rtot)
            nc.sync.dma_start(out=out_d[:], in_=out_t)

    _split_multi_waits(nc)
    return nc


_NC = None
last_results = None


def kernel(**inputs):
    global _NC, last_results
    _install_drain_patch()
    if _NC is None:
        _NC = _build()

    wqkv = np.empty((D, 7, D), np.float32)
    for i, nm in enumerate(("Wq3", "Wk", "Wv")):
        w = np.asarray(inputs[nm], np.float32)
        wqkv[:, 2 * i, :] = w[:D]
        wqkv[:, 2 * i + 1, :] = w[D:]
    wqkv[:, 6, :] = np.asarray(inputs["Wmhc"], np.float32)

    ed_base = np.zeros((D, EDATA_W), np.float32)
    ed_base[:, SW0] = np.asarray(inputs["b_mhc"], np.float32).reshape(D)
    ed_base[:, SW0 + 1] = np.asarray(inputs["Wshc"], np.float32).reshape(D)
    ed_base[0, SW0 + 2] = np.float32(np.asarray(inputs["b_shc"]).reshape(-1)[0])

    ejs = np.asarray(inputs["encoded_job"], np.float32)
    ems = np.asarray(inputs["encoded_machine"], np.float32)
    msks = np.asarray(inputs["ninf_mask"], np.float32)

    in_maps = []
    for b in range(B):
        ed = ed_base.copy()
        ed[0:J, EJ0 : EJ0 + D] = ejs[b]
        ed[0:M, EM0 : EM0 + D] = ems[b]
        ed[0:J, MK0 : MK0 + M] = msks[b]
        in_maps.append({"wqkv": wqkv, "edata": ed})

    last_results = run_bass_kernel_spmd(_NC, in_maps, core_ids=list(range(B)))
    out = np.stack(
        [last_results.results[b]["out"].reshape(J * M) for b in range(B)]
    )
    return out.astype(np.float32)



# revision 6
# speedup vs baseline: 1.4584x; 1.4584x over previous
"""FJSP decoder kernel for Trainium2, data-parallel over batch on 8 NeuronCores.

Factorized attention (see derivation in the module docstring of the
reference): for s=(j,m), q/k/v atoms decompose as x[s] = xj[j] + xm[m], so
the joint softmax splits into an E-side (contraction over j', K=100) and an
F-side (contraction over m', K=20), and the multi-head combine collapses
through w2 = Wmhc @ Wshc into per-head scalars uv = v @ w2:

  [SE|Nj] = eA @ [eC^T | eC^T*uvj];  [SF|Nm] = eB @ [eD^T | eD^T*uvm]
  score1  = sum_h Nj/SE + Nm/SF (+bias);  p = softmax(10*tanh(score1/sqrt(D)))

Host folds Wv/Wmhc/Wshc/b_* into wvfold [2D,8] + one bias scalar, transposes
ej/em, and pre-pads the q-side weights into the 2-head-per-32-partition
window layout (head 2t at lanes 32t+0:16 in qE, head 2t+1 at 32t+16:32 in
qO, partner lanes zero; k stays compact since only one matmul operand needs
zeroed pad lanes).  Everything ships as ONE bf16 DMA.  All PE matmuls run
bf16 (1 cyc/col).  Per-head exps are merged in pairs into [100,480] PSUM
tiles to amortize ACT fixed cost; E-side uv scales go to DVE, F-side to
GPSIMD.  The combine is two DVE divides + one reduce; the logit tail is
tanh -> (optional mask add) -> exp(+accum) -> ones-matmul total -> one
fused scalar divide.
"""

import math

import numpy as np
import ml_dtypes

import concourse.bass as bass
import concourse.mybir as mybir
import concourse.tile as tile
from concourse.bass_utils import run_bass_kernel_spmd

F32 = mybir.dt.float32
BF16 = mybir.dt.bfloat16
AF = mybir.ActivationFunctionType
OP = mybir.AluOpType
AX = mybir.AxisListType

D, H, QD = 128, 8, 16
B, J, M = 8, 100, 20
INV_SQ = 1.0 / math.sqrt(QD)  # 0.25
SD = math.sqrt(D)

# data column layout (bf16): padded q weights, compact k weights, transposed
# activations, folded v weights, bias
QEJ, QEM, QOJ, QOM, KJ, KM = 0, 128, 256, 384, 512, 640
EJ, EM, MK, WVJ, WVM, BI = 768, 868, 888, 908, 916, 924
NCOL = 925

# pairs of heads sharing one PSUM tile; even-head pairs first so they are
# gated only by the qE copy
PAIRS = [(0, 2), (4, 6), (1, 3), (5, 7)]

# ---------------------------------------------------------------------------
# gen3 walrus accepts one sync-wait per instruction. Tile's kernel-tail
# drain accumulates one wait per active logical processor on a single
# Drain: spread them across engines (parallel waiting). Tile's semaphore
# pass can also attach >1 wait to ordinary instructions: shed extras onto
# same-engine NoOps inserted right before the offender.
_PATCHED = False


def _install_drain_patch():
    global _PATCHED
    if _PATCHED:
        return
    from concourse.tile import ScopedClock, TileContext

    def _split_drain_and_barrier(self, tick_clock, wait_clock):
        drain_inst = self.nc.sync.drain()
        wait_clock.add_sem_waits(
            drain_inst.ins, ScopedClock({None: tick_clock.global_clock})
        )
        si = drain_inst.ins.sync_info
        waits = list(si.on_wait) if si is not None else []
        if len(waits) > 1:
            assert not si.on_update
            sems = {s.name: s for s in self.sems.allocated().values()}
            drain_inst.ins.sync_info = None
            drain_inst.wait_op(sems[waits[0].ant_name], waits[0].wait_value, "sem-ge")
            engines = [
                self.nc.scalar,
                self.nc.vector,
                self.nc.tensor,
                self.nc.gpsimd,
                self.nc.sync,
            ]
            for i, w in enumerate(waits[1:]):
                extra = engines[i % len(engines)].drain()
                extra.wait_op(sems[w.ant_name], w.wait_value, "sem-ge")
        self.nc.all_engine_barrier()
        assert self.sems is not None
        popped = self.nc._tile_sem_poison_stack.pop()
        assert popped is self._sem_poison
        self.nc.clear_and_free_semaphores(list(self.sems.allocated().values()))

    TileContext._drain_and_barrier = _split_drain_and_barrier
    _PATCHED = True


def _split_multi_waits(nc):
    import bass_rust

    ctr = 0
    for fn in nc.m.functions:
        for bb in fn.blocks:
            il = bb.instructions
            if not any(
                i.sync_info is not None and len(i.sync_info.on_wait) > 1 for i in il
            ):
                continue
            new = []
            for ins in il:
                si = ins.sync_info
                if si is not None and len(si.on_wait) > 1:
                    waits = list(si.on_wait)
                    ups = list(si.on_update)
                    for w in waits[:-1]:
                        nop = mybir.InstNoOp(name=f"I-waitsplit-{ctr}", ins=[], outs=[])
                        ctr += 1
                        nop.engine = ins.engine
                        nop.sync_info = bass_rust.SyncInfo(on_update=[], on_wait=[w])
                        new.append(nop)
                    ins.sync_info = bass_rust.SyncInfo(
                        on_update=ups, on_wait=[waits[-1]]
                    )
                new.append(ins)
            bb.instructions = new


def _build(with_mask: bool):
    nc = bass.Bass()
    data_d = nc.dram_tensor("data", [D, NCOL], BF16, kind="ExternalInput")
    out_d = nc.dram_tensor("out", [J, M], F32, kind="ExternalOutput")

    with tile.TileContext(nc) as tc:
        with (
            tc.tile_pool(name="persist", bufs=1) as pp,
            tc.tile_pool(name="rot", bufs=4) as rp,
            tc.tile_pool(name="ps_misc", bufs=2, space="PSUM") as ps_misc,
            tc.tile_pool(name="ps_pair", bufs=4, space="PSUM") as ps_pair,
            tc.tile_pool(name="ps_sf", bufs=2, space="PSUM") as ps_sf,
        ):
            ones_sb = pp.tile([D, J], F32, tag="ones")
            nc.gpsimd.memset(ones_sb, 1.0)

            data_sb = pp.tile([D, NCOL], BF16, tag="data")
            nc.sync.dma_start(out=data_sb, in_=data_d[:])

            ejT = data_sb[:, EJ : EJ + J]
            emT = data_sb[:, EM : EM + M]
            mask_v = data_sb[0:J, MK : MK + M]
            wvj_v = data_sb[:, WVJ : WVJ + H]
            wvm_v = data_sb[:, WVM : WVM + H]
            bias_v = data_sb[0:J, BI : BI + 1]

            # ---- projections: one PSUM tile, k first -------------------
            # cols 0:120 qE atoms, 120:240 qO atoms, 240:360 k atoms
            # (within each block: 0:100 job atoms, 100:120 machine atoms)
            pj_ps = ps_misc.tile([D, 360], F32, tag="misc")
            nc.tensor.matmul(
                out=pj_ps[:, 240:340], lhsT=data_sb[:, KJ : KJ + D], rhs=ejT
            )
            nc.tensor.matmul(
                out=pj_ps[:, 340:360], lhsT=data_sb[:, KM : KM + D], rhs=emT
            )
            nc.tensor.matmul(
                out=pj_ps[:, 0:100], lhsT=data_sb[:, QEJ : QEJ + D], rhs=ejT
            )
            nc.tensor.matmul(
                out=pj_ps[:, 100:120], lhsT=data_sb[:, QEM : QEM + D], rhs=emT
            )
            nc.tensor.matmul(
                out=pj_ps[:, 120:220], lhsT=data_sb[:, QOJ : QOJ + D], rhs=ejT
            )
            nc.tensor.matmul(
                out=pj_ps[:, 220:240], lhsT=data_sb[:, QOM : QOM + D], rhs=emT
            )

            # uv[j,h] = (ej @ Wvfold_j), uv[m,8+h] = (em @ Wvfold_m)
            uv_ps = ps_misc.tile([D, 16], F32, tag="misc")
            nc.tensor.matmul(out=uv_ps[0:J, 0:8], lhsT=ejT, rhs=wvj_v)
            nc.tensor.matmul(out=uv_ps[0:M, 8:16], lhsT=emT, rhs=wvm_v)

            pt = pp.tile([D, 360], BF16, tag="pt")
            nc.scalar.copy(out=pt[:, 240:360], in_=pj_ps[:, 240:360])  # kt on ACT
            nc.vector.tensor_copy(out=pt[:, 0:120], in_=pj_ps[:, 0:120])  # qE
            nc.vector.tensor_copy(out=pt[:, 120:240], in_=pj_ps[:, 120:240])  # qO
            kt = pt[:, 240:360]
            qE = pt[:, 0:120]
            qO = pt[:, 120:240]

            uv_sb = pp.tile([D, 16], F32, tag="uv")
            nc.vector.tensor_copy(out=uv_sb[0:J, :], in_=uv_ps[0:J, :])

            # ---- head loop: pair mms -> exp -> uv scales -> mm3/mm4 ----
            # per-head layout in e1 (stride 280):
            #   0:100 eA^T | 100:120 eC^T | 120:140 eC^T*uvj |
            #   140:240 eB^T | 240:260 eD^T | 260:280 eD^T*uvm
            pair_ps = []
            for ha, hb in PAIRS:
                ps = ps_pair.tile([D, 480], F32, tag="pair")
                for ci, h in enumerate((ha, hb)):
                    t = h // 2
                    qv = qE if h % 2 == 0 else qO
                    c0 = 240 * ci
                    nc.tensor.matmul(
                        out=ps[0:J, c0 : c0 + 120],
                        lhsT=kt[32 * t : 32 * t + 32, 0:100],
                        rhs=qv[32 * t : 32 * t + 32, 0:120],
                        tile_position=(32 * t, 0),
                    )
                    nc.tensor.matmul(
                        out=ps[0:M, c0 + 120 : c0 + 240],
                        lhsT=kt[32 * t : 32 * t + 32, 100:120],
                        rhs=qv[32 * t : 32 * t + 32, 0:120],
                        tile_position=(32 * t, 0),
                    )
                pair_ps.append(ps)

            e1s = []
            for p, (ha, hb) in enumerate(PAIRS):
                ps = pair_ps[p]
                e1 = rp.tile([D, 560], BF16, tag="e1")
                in_v = ps[0:J, 0:480].rearrange("p (a b x) -> p a b x", a=2, x=120)
                out_v = e1[0:J, 0:560].rearrange("p (a b y) -> p a b y", a=2, y=140)[
                    :, :, :, 0:120
                ]
                nc.scalar.activation(out=out_v, in_=in_v, func=AF.Exp, scale=INV_SQ)
                e1s.append(e1)
                for ci, h in enumerate((ha, hb)):
                    c0 = 280 * ci
                    nc.vector.tensor_scalar_mul(
                        out=e1[0:J, c0 + 120 : c0 + 140],
                        in0=e1[0:J, c0 + 100 : c0 + 120],
                        scalar1=uv_sb[0:J, h : h + 1],
                    )
                    nc.gpsimd.tensor_scalar_mul(
                        out=e1[0:M, c0 + 260 : c0 + 280],
                        in0=e1[0:M, c0 + 240 : c0 + 260],
                        scalar1=uv_sb[0:M, 8 + h : 9 + h],
                    )

            s_ps = ps_sf.tile([D, H, 40], F32, tag="sf")
            f_ps = ps_sf.tile([D, H, 40], F32, tag="sf")
            for p, (ha, hb) in enumerate(PAIRS):
                e1 = e1s[p]
                for ci, h in enumerate((ha, hb)):
                    c0 = 280 * ci
                    nc.tensor.matmul(
                        out=f_ps[0:J, h, :],
                        lhsT=e1[0:M, c0 + 140 : c0 + 240],
                        rhs=e1[0:M, c0 + 240 : c0 + 280],
                    )
                    nc.tensor.matmul(
                        out=s_ps[0:J, h, :],
                        lhsT=e1[0:J, c0 : c0 + 100],
                        rhs=e1[0:J, c0 + 100 : c0 + 140],
                    )

            # ---- combine: sum_h Nj/SE + Nm/SF --------------------------
            # DVE has no divide ALU op: reciprocal (one PSUM input) + mult
            rf = pp.tile([D, H, M], F32, tag="rf")
            nc.vector.reciprocal(out=rf[0:J, :, :], in_=f_ps[0:J, :, 0:M])
            rs = pp.tile([D, H, M], F32, tag="rs")
            nc.vector.reciprocal(out=rs[0:J, :, :], in_=s_ps[0:J, :, 0:M])
            dall = pp.tile([D, M, 2 * H], F32, tag="dall")
            nc.vector.tensor_tensor(
                out=dall[0:J, :, 0:8].rearrange("p m h -> p h m"),
                in0=f_ps[0:J, :, M : 2 * M],
                in1=rf[0:J, :, :],
                op=OP.mult,
            )
            nc.vector.tensor_tensor(
                out=dall[0:J, :, 8:16].rearrange("p m h -> p h m"),
                in0=s_ps[0:J, :, M : 2 * M],
                in1=rs[0:J, :, :],
                op=OP.mult,
            )
            c1 = pp.tile([D, M], F32, tag="c1")
            nc.vector.reduce_sum(out=c1[0:J, :], in_=dall[0:J, :, :], axis=AX.X)

            # ---- logits tail -------------------------------------------
            t_sb = pp.tile([D, M], F32, tag="t")
            nc.scalar.activation(
                out=t_sb[0:J, :], in_=c1[0:J, :], func=AF.Tanh,
                scale=1.0 / SD, bias=bias_v,
            )
            e_sb = pp.tile([J, M], F32, tag="e")
            s_row = pp.tile([J, 1], F32, tag="srow")
            if with_mask:
                arg = pp.tile([J, M], F32, tag="arg")
                nc.vector.scalar_tensor_tensor(
                    out=arg, in0=t_sb[0:J, :], scalar=10.0, in1=mask_v,
                    op0=OP.mult, op1=OP.add,
                )
                nc.scalar.activation(
                    out=e_sb, in_=arg, func=AF.Exp, scale=1.0, accum_out=s_row
                )
            else:
                nc.scalar.activation(
                    out=e_sb, in_=t_sb[0:J, :], func=AF.Exp, scale=10.0,
                    accum_out=s_row,
                )
            tot_ps = ps_misc.tile([D, 8], F32, tag="misc")
            nc.tensor.matmul(
                out=tot_ps[0:J, 0:1], lhsT=ones_sb[0:J, 0:J], rhs=s_row
            )
            rtot = pp.tile([J, 1], F32, tag="rtot")
            nc.vector.reciprocal(out=rtot, in_=tot_ps[0:J, 0:1])
            out_t = pp.tile([J, M], F32, tag="outt")
            nc.vector.tensor_scalar_mul(out=out_t, in0=e_sb, scalar1=rtot)
            nc.sync.dma_start(out=out_d[:], in_=out_t)

    _split_multi_waits(nc)
    return nc


_NC = None
_NC_MASKED = None
last_results = None


def _pack_weights(inputs):
    Wq3 = np.asarray(inputs["Wq3"], np.float32)
    Wk = np.asarray(inputs["Wk"], np.float32)
    Wv = np.asarray(inputs["Wv"], np.float32)
    Wmhc = np.asarray(inputs["Wmhc"], np.float32)
    Wshc = np.asarray(inputs["Wshc"], np.float32).reshape(D)
    b_mhc = np.asarray(inputs["b_mhc"], np.float32).reshape(D)
    b_shc = float(np.asarray(inputs["b_shc"]).reshape(-1)[0])

    w2 = Wmhc @ Wshc  # [128]
    bias_c = float(b_mhc @ Wshc + b_shc)
    wvf = (Wv * w2[None, :]).reshape(2 * D, H, QD).sum(-1)  # [256, 8]

    base = np.zeros((D, NCOL), np.float32)
    for t in range(4):
        he, ho = 2 * t, 2 * t + 1
        sl_e = slice(QD * he, QD * he + QD)
        sl_o = slice(QD * ho, QD * ho + QD)
        base[:, QEJ + 32 * t : QEJ + 32 * t + 16] = Wq3[:D, sl_e]
        base[:, QEM + 32 * t : QEM + 32 * t + 16] = Wq3[D:, sl_e]
        base[:, QOJ + 32 * t + 16 : QOJ + 32 * t + 32] = Wq3[:D, sl_o]
        base[:, QOM + 32 * t + 16 : QOM + 32 * t + 32] = Wq3[D:, sl_o]
    base[:, KJ : KJ + D] = Wk[:D]
    base[:, KM : KM + D] = Wk[D:]
    base[:, WVJ : WVJ + H] = wvf[:D]
    base[:, WVM : WVM + H] = wvf[D:]
    base[:, BI] = bias_c / SD
    return base


def kernel(**inputs):
    global _NC, _NC_MASKED, last_results
    _install_drain_patch()

    msks = np.asarray(inputs["ninf_mask"], np.float32)
    with_mask = bool(np.any(msks != 0.0))
    if with_mask:
        if _NC_MASKED is None:
            _NC_MASKED = _build(True)
        nc = _NC_MASKED
    else:
        if _NC is None:
            _NC = _build(False)
        nc = _NC

    base = _pack_weights(inputs)
    ejs = np.asarray(inputs["encoded_job"], np.float32)
    ems = np.asarray(inputs["encoded_machine"], np.float32)

    in_maps = []
    for b in range(B):
        d = base.copy()
        d[:, EJ : EJ + J] = ejs[b].T
        d[:, EM : EM + M] = ems[b].T
        d[0:J, MK : MK + M] = msks[b]
        in_maps.append({"data": d.astype(ml_dtypes.bfloat16)})

    last_results = run_bass_kernel_spmd(nc, in_maps, core_ids=list(range(B)))
    out = np.stack(
        [np.asarray(last_results.results[b]["out"]).reshape(J * M) for b in range(B)]
    )
    return out.astype(np.float32)
